# revision 26
# baseline (speedup 1.0000x reference)
"""Trainium2 Bass kernel for BANLayer (bilinear attention network layer).

Computation (per reference):
  v_ = relu(wn_linear(v));  q_ = relu(wn_linear(q))            # (B,NV,HK),(B,NQ,HK)
  att = einsum('hk,bvk,bqk->bhvq', h, v_, q_) + h_bias          # (B,8,NV,NQ)  [output]
  fusion = einsum('bvk,bhvq,bqk->bk', v_, att, q_)              # (B,HK)
  logits = avgpool_k3(fusion)*3 -> batchnorm(batch stats)       # (B,HD)       [output]

Strategy: data-parallel over batch (4 per core, 8 cores). bf16 matmuls with
fp32 PSUM accumulation. The head-summed attention A = sum_h att[b,h] is
computed as a 9th head (h_bar = sum_h h[h]). Fusion is computed K-major:
  S.T[k,q] = sum_v v_row[v,k] * A[v,q]   (PE; v_row = PE-transpose of v_T)
  fusion[k] = sum_q q_T[k,q] * S.T[k,q]  (DVE mul + free-axis reduce)
The kernel returns att_maps and the raw fusion vectors; the 3-wide avgpool
and BatchNorm (49K flops on (32,1536)) run on host. A device-side collective
for the BN batch stats was measured to throttle EVERY matmul in the NEFF
from 216 to 263 ns (collectives firmware active for the whole kernel), so
the all-reduce is deliberately avoided.
"""

import os
import sys

if "/opt/trn_rl_repo" not in sys.path:
    sys.path.insert(0, "/opt/trn_rl_repo")

import numpy as np
import ml_dtypes

import concourse.bass as bass
import concourse.mybir as mybir
import concourse.tile as tile
from concourse import bacc
from concourse.bass_utils import run_bass_kernel_spmd
from concourse.masks import make_identity

# Problem dims
B, NV, NQ = 32, 256, 512
VD, QD, HD, K, HOUT = 512, 512, 512, 3, 8
HK = HD * K  # 1536
BN_EPS = 1e-5

N_CORES = 8
NB = B // N_CORES  # 4 local batches per core
KC = HK // 128     # 12 chunks of contraction/feature dim
KV = VD // 128     # 4 chunks of VD/QD
MV = NV // 128     # 2 chunks of NV
NH = HOUT + 1      # 8 heads + summed "9th head"

F32 = mybir.dt.float32
BF16 = mybir.dt.bfloat16
AF = mybir.ActivationFunctionType
ALU = mybir.AluOpType

bf16 = ml_dtypes.bfloat16


def build_kernel():
    nc = bacc.Bacc()
    with tile.TileContext(nc) as tc:
        with (
            tc.tile_pool(name="dram", bufs=1, space="DRAM") as dram,
            tc.tile_pool(name="const", bufs=1) as const,
            tc.tile_pool(name="io", bufs=2) as io,
            tc.tile_pool(name="proj", bufs=2) as proj,
            tc.tile_pool(name="vhp", bufs=1) as vhp,
            tc.tile_pool(name="outp", bufs=4) as outp,
            tc.tile_pool(name="fus", bufs=1) as fusp,
            tc.tile_pool(name="scr", bufs=2) as scr,
            tc.tile_pool(name="mm", bufs=6, space="PSUM") as mmp,
            tc.tile_pool(name="tp", bufs=2, space="PSUM") as tpp,
        ):
            # ---- DRAM I/O ----
            vT_d = dram.tile([NB, VD, NV], BF16, kind="ExternalInput", name="vT", uniquify=False)
            qT_d = dram.tile([NB, QD, NQ], BF16, kind="ExternalInput", name="qT", uniquify=False)
            wvT_d = dram.tile([VD, HK], BF16, kind="ExternalInput", name="wvT", uniquify=False)
            wqT_d = dram.tile([QD, HK], BF16, kind="ExternalInput", name="wqT", uniquify=False)
            vb_d = dram.tile([128, KC], F32, kind="ExternalInput", name="vb", uniquify=False)
            qb_d = dram.tile([128, KC], F32, kind="ExternalInput", name="qb", uniquify=False)
            h_d = dram.tile([128, KC, NH], F32, kind="ExternalInput", name="ht", uniquify=False)
            hb_d = dram.tile([128, NH], F32, kind="ExternalInput", name="hbt", uniquify=False)

            att_d = dram.tile([NB, HOUT, NV, NQ], F32, kind="ExternalOutput", name="att_out", uniquify=False)
            fus_d = dram.tile([NB, HK], F32, kind="ExternalOutput", name="fus_out", uniquify=False)

            # ---- load constants into SBUF ----
            # First-needed data first, in small pieces, so the first projection
            # chain (wv[kv0] + vT[b0]) isn't bandwidth-starved by later loads.
            wv_sb = const.tile([128, KV, HK], BF16)
            wq_sb = const.tile([128, KV, HK], BF16)
            vT_t0 = io.tile([128, KV, NV], BF16, tag="vin")
            for s in range(4):
                nc.sync.dma_start(wv_sb[:, 0, s * 384:(s + 1) * 384],
                                  wvT_d[0:128, s * 384:(s + 1) * 384])
            for kv in range(KV):
                nc.sync.dma_start(vT_t0[:, kv, :],
                                  vT_d[0, kv * 128:(kv + 1) * 128, :])
            vb_sb = const.tile([128, KC], F32)
            nc.sync.dma_start(vb_sb[:], vb_d[:])
            qb_sb = const.tile([128, KC], F32)
            nc.sync.dma_start(qb_sb[:], qb_d[:])
            for kv in range(1, KV):
                nc.sync.dma_start(wv_sb[:, kv, :], wvT_d[kv * 128:(kv + 1) * 128, :])
            qT_t0 = io.tile([128, KV, NQ], BF16, tag="qin")
            for kv in range(KV):
                nc.sync.dma_start(qT_t0[:, kv, :],
                                  qT_d[0, kv * 128:(kv + 1) * 128, :])
            for kv in range(KV):
                nc.sync.dma_start(wq_sb[:, kv, :], wqT_d[kv * 128:(kv + 1) * 128, :])
            h_sb = const.tile([128, KC, NH], F32)
            nc.sync.dma_start(h_sb[:], h_d[:])
            hb_sb = const.tile([128, NH], F32)
            nc.sync.dma_start(hb_sb[:], hb_d[:])
            ident = const.tile([128, 128], BF16)
            make_identity(nc, ident)

            # fusion columns: fus_sb[p, b, kc] = fusion[b, kc*128+p]
            fus_sb = fusp.tile([128, NB, KC], F32)

            def st_fusion(pb, pvrow, pA, pq):
                # S.T[p, mk, q] = sum_v vrow[v, mk*128+p] * A[v, q]; then
                # fusion[pb, mk*128+p] = sum_q q_T[p, mk, q] * S.T[p, mk, q]
                for mk in range(KC):
                    ps = mmp.tile([128, NQ], F32, tag="mm", name=f"stps{pb}_{mk}")
                    for mv in range(MV):
                        nc.tensor.matmul(
                            ps[:], pvrow[:, mv, mk * 128:(mk + 1) * 128], pA[:, mv, :],
                            start=(mv == 0), stop=(mv == MV - 1))
                    prod = scr.tile([128, NQ], F32, tag="prod", name=f"prod{pb}_{mk}")
                    nc.vector.tensor_mul(prod[:], pq[:, mk, :], ps[:])
                    nc.vector.reduce_sum(fus_sb[:, pb, mk:mk + 1], prod[:], axis=mybir.AxisListType.X)
                nc.sync.dma_start(fus_d[pb].rearrange("(c p) -> p c", p=128), fus_sb[:, pb, :])

            for b in range(NB):
                if b == 0:
                    vT_t, qT_t = vT_t0, qT_t0
                else:
                    vT_t = io.tile([128, KV, NV], BF16, tag="vin")
                    nc.sync.dma_start(vT_t[:], vT_d[b].rearrange("(c p) n -> p c n", p=128))
                    qT_t = io.tile([128, KV, NQ], BF16, tag="qin")
                    nc.sync.dma_start(qT_t[:], qT_d[b].rearrange("(c p) n -> p c n", p=128))

                # K-major projections: v_sb[p, kc, n] = v_[b, n, kc*128+p]
                v_sb = proj.tile([128, KC, NV], BF16, tag="vsb")
                for m in range(KC):
                    ps = mmp.tile([128, NV], F32, tag="mm")
                    for kv in range(KV):
                        nc.tensor.matmul(
                            ps[:], wv_sb[:, kv, m * 128:(m + 1) * 128], vT_t[:, kv, :],
                            start=(kv == 0), stop=(kv == KV - 1))
                    nc.scalar.activation(v_sb[:, m, :], ps[:], AF.Relu, bias=vb_sb[:, m:m + 1])

                q_sb = proj.tile([128, KC, NQ], BF16, tag="qsb")
                for m in range(KC):
                    ps = mmp.tile([128, NQ], F32, tag="mm")
                    for kv in range(KV):
                        nc.tensor.matmul(
                            ps[:], wq_sb[:, kv, m * 128:(m + 1) * 128], qT_t[:, kv, :],
                            start=(kv == 0), stop=(kv == KV - 1))
                    nc.scalar.activation(q_sb[:, m, :], ps[:], AF.Relu, bias=qb_sb[:, m:m + 1])

                # v_row via PE transpose of v_T (identical post-relu bf16 values)
                vrow_sb = proj.tile([128, MV, HK], BF16, tag="vrow")
                for mv in range(MV):
                    for mk in range(KC):
                        tps = tpp.tile([128, 128], BF16, tag="tp")
                        nc.tensor.transpose(tps[:], v_sb[:, mk, mv * 128:(mv + 1) * 128], ident[:])
                        nc.scalar.activation(vrow_sb[:, mv, mk * 128:(mk + 1) * 128], tps[:],
                                             AF.Copy, bias=0.0)

                # vh[p, kc, j, n] = v_sb[p, kc, n] * h[j, kc*128+p]
                vh_sb = vhp.tile([128, KC, HOUT, NV], BF16, tag="vh")
                for hh in range(HOUT):
                    for kc in range(KC):
                        nc.vector.tensor_scalar_mul(
                            vh_sb[:, kc, hh, :], v_sb[:, kc, :], h_sb[:, kc, hh:hh + 1])

                if b > 0:
                    st_fusion(b - 1, prev[0], prev[1], prev[2])

                # attention maps; A = sum_h att[b,h] accumulated on DVE from the
                # fp32 head outputs (cheaper than a 9th head on PE, and exact)
                A_f32 = proj.tile([128, MV, NQ], F32, tag="Af32")
                A_sb = proj.tile([128, MV, NQ], BF16, tag="Asb")
                for hh in range(HOUT):
                    for m in range(MV):
                        ps = mmp.tile([128, NQ], F32, tag="mm")
                        for kc in range(KC):
                            nc.tensor.matmul(
                                ps[:], vh_sb[:, kc, hh, m * 128:(m + 1) * 128], q_sb[:, kc, :],
                                start=(kc == 0), stop=(kc == KC - 1))
                        ao = outp.tile([128, NQ], F32, tag="attout")
                        nc.scalar.activation(ao[:], ps[:], AF.Identity, bias=hb_sb[:, hh:hh + 1])
                        nc.sync.dma_start(att_d[b, hh, m * 128:(m + 1) * 128, :], ao[:])
                        if hh == 0:
                            nc.vector.tensor_copy(A_f32[:, m, :], ao[:])
                        elif hh < HOUT - 1:
                            nc.vector.tensor_add(A_f32[:, m, :], A_f32[:, m, :], ao[:])
                        else:
                            nc.vector.tensor_add(A_sb[:, m, :], A_f32[:, m, :], ao[:])

                prev = (vrow_sb, A_sb, q_sb)

            st_fusion(NB - 1, prev[0], prev[1], prev[2])


    nc.compile()
    return nc


def prep_inputs(v, q, v_V, v_g, v_b, q_V, q_g, q_b, h_mat, h_bias, bn_gamma, bn_beta):
    """Host-side prep: weight-norm fold, transposes, layout, bf16 casts.
    Returns per-core input maps."""
    wv = (v_V * (np.float32(v_g) / np.linalg.norm(v_V))).astype(np.float32)
    wq = (q_V * (np.float32(q_g) / np.linalg.norm(q_V))).astype(np.float32)
    h = h_mat[0, :, 0, :].astype(np.float32)          # (8, HK)
    hb = h_bias[0, :, 0, 0].astype(np.float32)        # (8,)
    h9 = np.concatenate([h, h.sum(0, keepdims=True)], 0)       # (9, HK)
    hb9 = np.concatenate([hb, hb.sum(keepdims=True)], 0)       # (9,)

    wvT = np.ascontiguousarray(wv.T).astype(bf16)              # (VD, HK)
    wqT = np.ascontiguousarray(wq.T).astype(bf16)
    vb_t = np.ascontiguousarray(v_b.reshape(KC, 128).T).astype(np.float32)
    qb_t = np.ascontiguousarray(q_b.reshape(KC, 128).T).astype(np.float32)
    # h_t[p, kc, j] = h9[j, kc*128+p]
    h_t = np.ascontiguousarray(h9.T.reshape(KC, 128, NH).transpose(1, 0, 2)).astype(np.float32)
    hb_t = np.broadcast_to(hb9, (128, NH)).copy().astype(np.float32)

    shared = {"wvT": wvT, "wqT": wqT, "vb": vb_t, "qb": qb_t, "ht": h_t, "hbt": hb_t}
    in_maps = []
    for c in range(N_CORES):
        sl = slice(c * NB, (c + 1) * NB)
        vT = np.ascontiguousarray(v[sl].transpose(0, 2, 1)).astype(bf16)
        qT = np.ascontiguousarray(q[sl].transpose(0, 2, 1)).astype(bf16)
        in_maps.append({"vT": vT, "qT": qT, **shared})
    return in_maps


_NC_CACHE = None


def _get_nc():
    global _NC_CACHE
    if _NC_CACHE is None:
        _NC_CACHE = build_kernel()
    return _NC_CACHE


def kernel(v, q, v_V, v_g, v_b, q_V, q_g, q_b, h_mat, h_bias, bn_gamma, bn_beta,
           _trace=False, _trace_kwargs=None):
    v, q = np.asarray(v), np.asarray(q)
    v_V, v_b, q_V, q_b = map(np.asarray, (v_V, v_b, q_V, q_b))
    h_mat, h_bias = np.asarray(h_mat), np.asarray(h_bias)
    bn_gamma, bn_beta = np.asarray(bn_gamma), np.asarray(bn_beta)
    nc = _get_nc()
    in_maps = prep_inputs(v, q, v_V, v_g, v_b, q_V, q_g, q_b, h_mat, h_bias,
                          bn_gamma, bn_beta)
    res = run_bass_kernel_spmd(nc, in_maps, list(range(N_CORES)), trace=_trace,
                               **(_trace_kwargs or {}))
    kernel.last_results = res
    fusion = np.empty((B, HK), np.float32)
    att = np.empty((B, HOUT, NV, NQ), np.float32)
    for c in range(N_CORES):
        sl = slice(c * NB, (c + 1) * NB)
        fusion[sl] = res.results[c]["fus_out"]
        att[sl] = res.results[c]["att_out"]
    # avgpool(k=3)*3 + BatchNorm (train-mode batch stats): 49K flops on host
    logits = fusion.reshape(B, HD, K).sum(-1)
    mu = logits.mean(0)
    var = np.mean((logits - mu) ** 2, axis=0)
    logits = ((logits - mu) / np.sqrt(var + BN_EPS) * bn_gamma + bn_beta).astype(np.float32)
    return logits, att


# revision 28
# speedup vs baseline: 1.0526x; 1.0526x over previous
"""Trainium2 Bass kernel for BANLayer (bilinear attention network layer).

Computation (per reference):
  v_ = relu(wn_linear(v));  q_ = relu(wn_linear(q))            # (B,NV,HK),(B,NQ,HK)
  att = einsum('hk,bvk,bqk->bhvq', h, v_, q_) + h_bias          # (B,8,NV,NQ)  [output]
  fusion = einsum('bvk,bhvq,bqk->bk', v_, att, q_)              # (B,HK)
  logits = avgpool_k3(fusion)*3 -> batchnorm(batch stats)       # (B,HD)       [output]

Strategy: data-parallel over batch (4 per core, 8 cores). bf16 matmuls with
fp32 PSUM accumulation. The head-summed attention A = sum_h att[b,h] is
computed as a 9th head (h_bar = sum_h h[h]). Fusion is computed K-major:
  S.T[k,q] = sum_v v_row[v,k] * A[v,q]   (PE; v_row = PE-transpose of v_T)
  fusion[k] = sum_q q_T[k,q] * S.T[k,q]  (DVE mul + free-axis reduce)
The kernel returns att_maps and the raw fusion vectors; the 3-wide avgpool
and BatchNorm (49K flops on (32,1536)) run on host. A device-side collective
for the BN batch stats was measured to throttle EVERY matmul in the NEFF
from 216 to 263 ns (collectives firmware active for the whole kernel), so
the all-reduce is deliberately avoided.
"""

import os
import sys

if "/opt/trn_rl_repo" not in sys.path:
    sys.path.insert(0, "/opt/trn_rl_repo")

import numpy as np
import ml_dtypes

import concourse.bass as bass
import concourse.mybir as mybir
import concourse.tile as tile
from concourse import bacc
from concourse.bass_utils import run_bass_kernel_spmd
from concourse.masks import make_identity

# Problem dims
B, NV, NQ = 32, 256, 512
VD, QD, HD, K, HOUT = 512, 512, 512, 3, 8
HK = HD * K  # 1536
BN_EPS = 1e-5

N_CORES = 8
NB = B // N_CORES  # 4 local batches per core
KC = HK // 128     # 12 chunks of contraction/feature dim
KV = VD // 128     # 4 chunks of VD/QD
MV = NV // 128     # 2 chunks of NV
NH = HOUT + 1      # 8 heads + summed "9th head"

F32 = mybir.dt.float32
BF16 = mybir.dt.bfloat16
AF = mybir.ActivationFunctionType
ALU = mybir.AluOpType

bf16 = ml_dtypes.bfloat16


def build_kernel():
    nc = bacc.Bacc()
    with tile.TileContext(nc) as tc:
        with (
            tc.tile_pool(name="dram", bufs=1, space="DRAM") as dram,
            tc.tile_pool(name="const", bufs=1) as const,
            tc.tile_pool(name="io", bufs=2) as io,
            tc.tile_pool(name="proj", bufs=2) as proj,
            tc.tile_pool(name="vhp", bufs=1) as vhp,
            tc.tile_pool(name="outp", bufs=4) as outp,
            tc.tile_pool(name="fus", bufs=1) as fusp,
            tc.tile_pool(name="scr", bufs=2) as scr,
            tc.tile_pool(name="stp", bufs=1) as stp,
            tc.tile_pool(name="mm", bufs=6, space="PSUM") as mmp,
            tc.tile_pool(name="tp", bufs=2, space="PSUM") as tpp,
        ):
            # ---- DRAM I/O ----
            vT_d = dram.tile([NB, VD, NV], BF16, kind="ExternalInput", name="vT", uniquify=False)
            qT_d = dram.tile([NB, QD, NQ], BF16, kind="ExternalInput", name="qT", uniquify=False)
            wvT_d = dram.tile([VD, HK], BF16, kind="ExternalInput", name="wvT", uniquify=False)
            wqT_d = dram.tile([QD, HK], BF16, kind="ExternalInput", name="wqT", uniquify=False)
            vb_d = dram.tile([128, KC], F32, kind="ExternalInput", name="vb", uniquify=False)
            qb_d = dram.tile([128, KC], F32, kind="ExternalInput", name="qb", uniquify=False)
            h_d = dram.tile([128, KC, NH], F32, kind="ExternalInput", name="ht", uniquify=False)
            hb_d = dram.tile([128, NH], F32, kind="ExternalInput", name="hbt", uniquify=False)

            att_d = dram.tile([NB, HOUT, NV, NQ], F32, kind="ExternalOutput", name="att_out", uniquify=False)
            fus_d = dram.tile([NB, HK], F32, kind="ExternalOutput", name="fus_out", uniquify=False)

            # ---- load constants into SBUF ----
            # First-needed data first, in small pieces, so the first projection
            # chain (wv[kv0] + vT[b0]) isn't bandwidth-starved by later loads.
            wv_sb = const.tile([128, KV, HK], BF16)
            wq_sb = const.tile([128, KV, HK], BF16)
            vT_t0 = io.tile([128, KV, NV], BF16, tag="vin")
            for s in range(4):
                nc.sync.dma_start(wv_sb[:, 0, s * 384:(s + 1) * 384],
                                  wvT_d[0:128, s * 384:(s + 1) * 384])
            for kv in range(KV):
                nc.sync.dma_start(vT_t0[:, kv, :],
                                  vT_d[0, kv * 128:(kv + 1) * 128, :])
            vb_sb = const.tile([128, KC], F32)
            nc.sync.dma_start(vb_sb[:], vb_d[:])
            qb_sb = const.tile([128, KC], F32)
            nc.sync.dma_start(qb_sb[:], qb_d[:])
            for kv in range(1, KV):
                nc.sync.dma_start(wv_sb[:, kv, :], wvT_d[kv * 128:(kv + 1) * 128, :])
            qT_t0 = io.tile([128, KV, NQ], BF16, tag="qin")
            for kv in range(KV):
                nc.sync.dma_start(qT_t0[:, kv, :],
                                  qT_d[0, kv * 128:(kv + 1) * 128, :])
            for kv in range(KV):
                nc.sync.dma_start(wq_sb[:, kv, :], wqT_d[kv * 128:(kv + 1) * 128, :])
            h_sb = const.tile([128, KC, NH], F32)
            nc.sync.dma_start(h_sb[:], h_d[:])
            hb_sb = const.tile([128, NH], F32)
            nc.sync.dma_start(hb_sb[:], hb_d[:])
            ident = const.tile([128, 128], BF16)
            make_identity(nc, ident)

            # fusion columns: fus_sb[p, b, kc] = fusion[b, kc*128+p]
            fus_sb = fusp.tile([128, NB, KC], F32)

            def flush_fusion(pb, pq, pst):
                # fusion[pb, mk*128+p] = sum_q q_T[p, mk, q] * S.T[p, mk, q]
                for mk in range(KC):
                    prod = scr.tile([128, NQ], F32, tag="prod", name=f"prod{pb}_{mk}")
                    nc.vector.tensor_mul(prod[:], pq[:, mk, :], pst[:, mk, :])
                    nc.vector.reduce_sum(fus_sb[:, pb, mk:mk + 1], prod[:], axis=mybir.AxisListType.X)
                nc.sync.dma_start(fus_d[pb].rearrange("(c p) -> p c", p=128), fus_sb[:, pb, :])

            for b in range(NB):
                if b == 0:
                    vT_t, qT_t = vT_t0, qT_t0
                else:
                    vT_t = io.tile([128, KV, NV], BF16, tag="vin")
                    nc.sync.dma_start(vT_t[:], vT_d[b].rearrange("(c p) n -> p c n", p=128))
                    qT_t = io.tile([128, KV, NQ], BF16, tag="qin")
                    nc.sync.dma_start(qT_t[:], qT_d[b].rearrange("(c p) n -> p c n", p=128))

                # K-major projections: v_sb[p, kc, n] = v_[b, n, kc*128+p]
                v_sb = proj.tile([128, KC, NV], BF16, tag="vsb")
                for m in range(KC):
                    ps = mmp.tile([128, NV], F32, tag="mm")
                    for kv in range(KV):
                        nc.tensor.matmul(
                            ps[:], wv_sb[:, kv, m * 128:(m + 1) * 128], vT_t[:, kv, :],
                            start=(kv == 0), stop=(kv == KV - 1))
                    nc.scalar.activation(v_sb[:, m, :], ps[:], AF.Relu, bias=vb_sb[:, m:m + 1])

                q_sb = proj.tile([128, KC, NQ], BF16, tag="qsb")
                for m in range(KC):
                    ps = mmp.tile([128, NQ], F32, tag="mm")
                    for kv in range(KV):
                        nc.tensor.matmul(
                            ps[:], wq_sb[:, kv, m * 128:(m + 1) * 128], qT_t[:, kv, :],
                            start=(kv == 0), stop=(kv == KV - 1))
                    nc.scalar.activation(q_sb[:, m, :], ps[:], AF.Relu, bias=qb_sb[:, m:m + 1])

                # v_row via PE transpose of v_T (identical post-relu bf16 values)
                vrow_sb = proj.tile([128, MV, HK], BF16, tag="vrow")
                for mv in range(MV):
                    for mk in range(KC):
                        tps = tpp.tile([128, 128], BF16, tag="tp")
                        nc.tensor.transpose(tps[:], v_sb[:, mk, mv * 128:(mv + 1) * 128], ident[:])
                        nc.scalar.activation(vrow_sb[:, mv, mk * 128:(mk + 1) * 128], tps[:],
                                             AF.Copy, bias=0.0)

                # vh[p, kc, j, n] = v_sb[p, kc, n] * h[j, kc*128+p]
                vh_sb = vhp.tile([128, KC, HOUT, NV], BF16, tag="vh")
                for hh in range(HOUT):
                    for kc in range(KC):
                        nc.vector.tensor_scalar_mul(
                            vh_sb[:, kc, hh, :], v_sb[:, kc, :], h_sb[:, kc, hh:hh + 1])

                if b > 0:
                    flush_fusion(*pend)

                # attention maps; A = sum_h att[b,h] accumulated on DVE from the
                # fp32 head outputs (cheaper than a 9th head on PE, and exact)
                A_f32 = proj.tile([128, MV, NQ], F32, tag="Af32")
                A_sb = proj.tile([128, MV, NQ], BF16, tag="Asb")
                for hh in range(HOUT):
                    for m in range(MV):
                        ps = mmp.tile([128, NQ], F32, tag="mm")
                        for kc in range(KC):
                            nc.tensor.matmul(
                                ps[:], vh_sb[:, kc, hh, m * 128:(m + 1) * 128], q_sb[:, kc, :],
                                start=(kc == 0), stop=(kc == KC - 1))
                        ao = outp.tile([128, NQ], F32, tag="attout")
                        nc.scalar.activation(ao[:], ps[:], AF.Identity, bias=hb_sb[:, hh:hh + 1])
                        nc.sync.dma_start(att_d[b, hh, m * 128:(m + 1) * 128, :], ao[:])
                        if hh == 0:
                            nc.vector.tensor_copy(A_f32[:, m, :], ao[:])
                        elif hh < HOUT - 1:
                            nc.vector.tensor_add(A_f32[:, m, :], A_f32[:, m, :], ao[:])
                        else:
                            nc.vector.tensor_add(A_sb[:, m, :], A_f32[:, m, :], ao[:])

                # S.T[p, mk, q] = sum_v vrow[v, mk*128+p] * A[v, q]
                # PSUM evicted to SBUF at PE pace (ACT); the fusion DVE work is
                # deferred into the next batch's attention window.
                ST_sb = stp.tile([128, KC, NQ], F32, tag="ST")
                for mk in range(KC):
                    ps = mmp.tile([128, NQ], F32, tag="mm")
                    for mv in range(MV):
                        nc.tensor.matmul(
                            ps[:], vrow_sb[:, mv, mk * 128:(mk + 1) * 128], A_sb[:, mv, :],
                            start=(mv == 0), stop=(mv == MV - 1))
                    nc.scalar.activation(ST_sb[:, mk, :], ps[:], AF.Copy, bias=0.0)
                pend = (b, q_sb, ST_sb)

            flush_fusion(*pend)

    nc.compile()
    return nc


def prep_inputs(v, q, v_V, v_g, v_b, q_V, q_g, q_b, h_mat, h_bias, bn_gamma, bn_beta):
    """Host-side prep: weight-norm fold, transposes, layout, bf16 casts.
    Returns per-core input maps."""
    wv = (v_V * (np.float32(v_g) / np.linalg.norm(v_V))).astype(np.float32)
    wq = (q_V * (np.float32(q_g) / np.linalg.norm(q_V))).astype(np.float32)
    h = h_mat[0, :, 0, :].astype(np.float32)          # (8, HK)
    hb = h_bias[0, :, 0, 0].astype(np.float32)        # (8,)
    h9 = np.concatenate([h, h.sum(0, keepdims=True)], 0)       # (9, HK)
    hb9 = np.concatenate([hb, hb.sum(keepdims=True)], 0)       # (9,)

    wvT = np.ascontiguousarray(wv.T).astype(bf16)              # (VD, HK)
    wqT = np.ascontiguousarray(wq.T).astype(bf16)
    vb_t = np.ascontiguousarray(v_b.reshape(KC, 128).T).astype(np.float32)
    qb_t = np.ascontiguousarray(q_b.reshape(KC, 128).T).astype(np.float32)
    # h_t[p, kc, j] = h9[j, kc*128+p]
    h_t = np.ascontiguousarray(h9.T.reshape(KC, 128, NH).transpose(1, 0, 2)).astype(np.float32)
    hb_t = np.broadcast_to(hb9, (128, NH)).copy().astype(np.float32)

    shared = {"wvT": wvT, "wqT": wqT, "vb": vb_t, "qb": qb_t, "ht": h_t, "hbt": hb_t}
    in_maps = []
    for c in range(N_CORES):
        sl = slice(c * NB, (c + 1) * NB)
        vT = np.ascontiguousarray(v[sl].transpose(0, 2, 1)).astype(bf16)
        qT = np.ascontiguousarray(q[sl].transpose(0, 2, 1)).astype(bf16)
        in_maps.append({"vT": vT, "qT": qT, **shared})
    return in_maps


_NC_CACHE = None


def _get_nc():
    global _NC_CACHE
    if _NC_CACHE is None:
        _NC_CACHE = build_kernel()
    return _NC_CACHE


def kernel(v, q, v_V, v_g, v_b, q_V, q_g, q_b, h_mat, h_bias, bn_gamma, bn_beta,
           _trace=False, _trace_kwargs=None):
    v, q = np.asarray(v), np.asarray(q)
    v_V, v_b, q_V, q_b = map(np.asarray, (v_V, v_b, q_V, q_b))
    h_mat, h_bias = np.asarray(h_mat), np.asarray(h_bias)
    bn_gamma, bn_beta = np.asarray(bn_gamma), np.asarray(bn_beta)
    nc = _get_nc()
    in_maps = prep_inputs(v, q, v_V, v_g, v_b, q_V, q_g, q_b, h_mat, h_bias,
                          bn_gamma, bn_beta)
    res = run_bass_kernel_spmd(nc, in_maps, list(range(N_CORES)), trace=_trace,
                               **(_trace_kwargs or {}))
    kernel.last_results = res
    fusion = np.empty((B, HK), np.float32)
    att = np.empty((B, HOUT, NV, NQ), np.float32)
    for c in range(N_CORES):
        sl = slice(c * NB, (c + 1) * NB)
        fusion[sl] = res.results[c]["fus_out"]
        att[sl] = res.results[c]["att_out"]
    # avgpool(k=3)*3 + BatchNorm (train-mode batch stats): 49K flops on host
    logits = fusion.reshape(B, HD, K).sum(-1)
    mu = logits.mean(0)
    var = np.mean((logits - mu) ** 2, axis=0)
    logits = ((logits - mu) / np.sqrt(var + BN_EPS) * bn_gamma + bn_beta).astype(np.float32)
    return logits, att
